# revision 18
# baseline (speedup 1.0000x reference)
"""Haar-DWT downsampling + 1x1 conv + BN + ReLU fused Trainium2 kernel.

Math: the Haar DWT (J=1) followed by a 1x1 conv over the 4C subband
channels, inference BN, and ReLU is one linear op + bias + ReLU.  It
folds into a 2x2/stride-2 conv:

    z[o, i, j] = relu( sum_{c,di,dj} Weff[o, c, di, dj] * x[c, 2i+di, 2j+dj]
                       + bias_total[o] )

with Weff/bias_total computed on the host from (W, b, gamma, beta, mean,
var).  On-device this is, per output tile, 2 accumulating matmuls
(contraction K = 128 = (c, di), one per dj) + one scalar-engine
activation (bias + ReLU) reading PSUM.

Sharding: pure data-parallel over batch. B=16 -> 2 images per core on
8 cores. Each core reads only its x shard and writes only its z shard
(minimal HBM traffic: 33.5 MB in + 16.8 MB out per core).
"""

import numpy as np

import concourse.bass as bass
import concourse.bacc as bacc
import concourse.mybir as mybir
from concourse.tile import TileContext
from concourse.bass_utils import run_bass_kernel_spmd

BN_EPS = 1e-5

# Problem shape (hardcoded per harness contract)
B, C, H, W_IMG = 16, 64, 256, 256
COUT = 128
N_CORES = 8
B_LOCAL = B // N_CORES          # 2 images per core
HO, WO = H // 2, W_IMG // 2     # 128 x 128 output image

NROWS = 16                      # output rows per tile
N_ROW_BLOCKS = HO // NROWS      # 8
GROUPS = (NROWS * WO) // 512    # matmul free-dim groups of 512

F32 = mybir.dt.float32
F32R = mybir.dt.float32r


def _fold_weights(W, b, gamma, beta, mean, var):
    """Fold DWT + conv + BN into lhsT weights [2(dj), 128(K=(c,di)), 128(M=o)]
    and a per-channel bias [COUT]."""
    W = W.astype(np.float64)
    Wll, Wlh, Whl, Whh = W[:, :C], W[:, C:2 * C], W[:, 2 * C:3 * C], W[:, 3 * C:]
    s = (gamma.astype(np.float64) / np.sqrt(var.astype(np.float64) + BN_EPS))
    coef = {
        (0, 0): 0.5 * (Wll + Wlh + Whl + Whh),
        (0, 1): 0.5 * (Wll + Wlh - Whl - Whh),
        (1, 0): 0.5 * (Wll - Wlh + Whl - Whh),
        (1, 1): 0.5 * (Wll - Wlh - Whl + Whh),
    }
    bias_total = (b.astype(np.float64) * s + beta.astype(np.float64)
                  - mean.astype(np.float64) * s)
    # lhsT_dj[k, o] with k = di*64 + c; scaled by BN scale s[o]
    lhsT = np.zeros((2, 128, COUT), dtype=np.float64)
    for dj in range(2):
        for di in range(2):
            lhsT[dj, di * C:(di + 1) * C, :] = (coef[(di, dj)] * s[:, None]).T
    return lhsT.astype(np.float32), bias_total.astype(np.float32)


def build_nc(b_local=B_LOCAL, n_row_blocks=N_ROW_BLOCKS, use_f32r=True,
             run_bacc_compile=True):
    nc = bacc.Bacc(None)
    mm_dt = F32R if use_f32r else F32
    x = nc.dram_tensor("x", [b_local, C, H, W_IMG], mm_dt, kind="ExternalInput")
    w_lhsT = nc.dram_tensor("w_lhsT", [2, 128, COUT], mm_dt, kind="ExternalInput")
    bias = nc.dram_tensor("bias", [COUT, 1], F32, kind="ExternalInput")
    z = nc.dram_tensor("z", [b_local, COUT, HO, WO], F32, kind="ExternalOutput")

    with TileContext(nc) as tc:
        with (
            tc.tile_pool(name="consts", bufs=1) as cpool,
            tc.tile_pool(name="xin", bufs=3) as xpool,
            tc.tile_pool(name="psum", bufs=2, space="PSUM") as ppool,
            tc.tile_pool(name="zout", bufs=3) as zpool,
        ):
            w0_sb = cpool.tile([128, COUT], mm_dt)
            w1_sb = cpool.tile([128, COUT], mm_dt)
            w_sb = [w0_sb, w1_sb]
            for dj in range(2):
                nc.sync.dma_start(out=w_sb[dj][:], in_=w_lhsT[dj])
            bias_sb = cpool.tile([COUT, 1], F32)
            nc.sync.dma_start(out=bias_sb[:], in_=bias[:])

            for bi in range(b_local):
                for rb in range(n_row_blocks):
                    r0 = 2 * NROWS * rb  # first input row of this tile
                    xt = xpool.tile([128, NROWS * W_IMG], mm_dt)
                    # partition = (di, c): p = di*64 + c ; free = (il, w).
                    # One DMA per di: each hits a disjoint 64-partition half
                    # (disjoint SBUF port sets), so the two run concurrently.
                    for di in range(2):
                        src = x[bi, :, r0 + di:r0 + 2 * NROWS:2, :]
                        nc.sync.dma_start(
                            out=xt[di * C:(di + 1) * C].rearrange(
                                "c (il w) -> c il w", w=W_IMG
                            ),
                            in_=src,
                        )

                    ps = ppool.tile([COUT, NROWS * WO], F32)
                    # free index of xt: il*256 + w, with w = 2*j + dj
                    xv = xt.rearrange("p (il j dj) -> p dj il j", j=WO, dj=2)
                    for dj in range(2):
                        for g in range(GROUPS):
                            nc.tensor.matmul(
                                ps[:, 512 * g:512 * (g + 1)],
                                lhsT=w_sb[dj][:],
                                rhs=xv[:, dj, 4 * g:4 * (g + 1), :],
                                start=(dj == 0),
                                stop=(dj == 1),
                            )

                    zt = zpool.tile([COUT, NROWS * WO], F32)
                    # bias + ReLU in one DVE pass: max(ps + bias, 0).
                    # (ACT measured ~6.9us per tile for this; DVE ~2.1us.)
                    nc.vector.tensor_scalar(
                        zt[:], ps[:], bias_sb[:, 0:1], 0.0,
                        mybir.AluOpType.add, mybir.AluOpType.max,
                    )
                    nc.sync.dma_start(
                        out=z[bi, :, NROWS * rb:NROWS * (rb + 1), :].rearrange(
                            "o i j -> o (i j)"
                        ),
                        in_=zt[:],
                    )
    if run_bacc_compile:
        nc.compile()
    return nc


_NC_CACHE = {}


def _get_nc():
    if "nc" not in _NC_CACHE:
        _NC_CACHE["nc"] = build_nc()
    return _NC_CACHE["nc"]


def kernel(x, W, b, gamma, beta, mean, var, _trace=False):
    x = np.ascontiguousarray(np.asarray(x, dtype=np.float32))
    lhsT, bias_total = _fold_weights(
        np.asarray(W), np.asarray(b), np.asarray(gamma),
        np.asarray(beta), np.asarray(mean), np.asarray(var),
    )
    bias_col = np.ascontiguousarray(bias_total.reshape(COUT, 1))

    nc = _get_nc()
    in_maps = []
    for core in range(N_CORES):
        xs = np.ascontiguousarray(x[core * B_LOCAL:(core + 1) * B_LOCAL])
        in_maps.append({"x": xs, "w_lhsT": lhsT, "bias": bias_col})

    res = run_bass_kernel_spmd(
        nc, in_maps, list(range(N_CORES)), trace=_trace
    )
    out = np.concatenate([res.results[i]["z"] for i in range(N_CORES)], axis=0)
    if _trace:
        return out, res
    return out
